# revision 4
# baseline (speedup 1.0000x reference)
"""Haar 2D DWT (pywt 'haar') Trainium2 Bass kernel.

Full input x: [16, 64, 256, 256] f32.
Output: [16, 256, 128, 128] f32 = concat(ll, lh, hl, hh) on channel axis.

Strategy: pure data-parallel shard over batch (16 -> 2 per core x 8 cores).
Per core, per (batch, channel-block) tile:
  - partition dim = row-pair index (H/2 = 128 partitions), each partition
    holds 2 consecutive image rows = 512 contiguous f32 (2KB DMA lines).
  - stage 0: scale whole tile by 0.5 (DVE tensor_scalar, 2x_2P f32 mode)
  - stage 1 (row butterfly): s = even_row + odd_row, d = even_row - odd_row
  - stage 2 (col butterfly): ll = s_e + s_o, lh = d_e + d_o,
                             hl = s_e - s_o, hh = d_e - d_o
  - one fused output DMA writes all four quadrant channel blocks.
"""

import numpy as np

N_CORES = 8
FULL_B, C, H, W = 16, 64, 256, 256


def _build_bass(B=2, Cc=64, Hh=256, Ww=256, G=8, bufs=2):
    import concourse.bacc as bacc
    import concourse.mybir as mybir
    from concourse.tile import TileContext

    P = Hh // 2          # partitions = row pairs
    Wh = Ww // 2
    f32 = mybir.dt.float32

    nc = bacc.Bacc("TRN2", target_bir_lowering=False, debug=False)
    x = nc.dram_tensor("x", [B, Cc, Hh, Ww], f32, kind="ExternalInput").ap()
    y = nc.dram_tensor("y", [B, 4 * Cc, P, Wh], f32, kind="ExternalOutput").ap()

    with TileContext(nc) as tc:
        with tc.tile_pool(name="pool", bufs=bufs) as pool:
            for b in range(B):
                # [P, C, 2*W]: partition = row pair, free = (channel, 2 rows)
                xb = x[b].rearrange("c (p t) w -> p c (t w)", t=2)
                # [P, 4, C, W/2]: (quadrant, channel, out col)
                yb = y[b].rearrange("(q c) p w -> p q c w", q=4)
                for c0 in range(0, Cc, G):
                    in_t = pool.tile([P, G * 2 * Ww], f32, tag="in")
                    nc.sync.dma_start(
                        out=in_t[:].rearrange("p (c e) -> p c e", c=G),
                        in_=xb[:, c0 : c0 + G, :],
                    )
                    nc.vector.tensor_scalar_mul(in_t[:], in_t[:], 0.5)
                    iv = in_t[:].rearrange("p (c t w) -> p c t w", c=G, t=2)
                    s_t = pool.tile([P, G * Ww], f32, tag="s")
                    d_t = pool.tile([P, G * Ww], f32, tag="d")
                    sv = s_t[:].rearrange("p (c w) -> p c w", c=G)
                    dv = d_t[:].rearrange("p (c w) -> p c w", c=G)
                    nc.vector.tensor_add(out=sv, in0=iv[:, :, 0, :], in1=iv[:, :, 1, :])
                    nc.vector.tensor_sub(out=dv, in0=iv[:, :, 0, :], in1=iv[:, :, 1, :])
                    out_t = pool.tile([P, 4 * G * Wh], f32, tag="out")
                    ov = out_t[:].rearrange("p (q c w) -> p q c w", q=4, c=G)
                    sp = s_t[:].rearrange("p (c w t) -> p c w t", c=G, t=2)
                    dp = d_t[:].rearrange("p (c w t) -> p c w t", c=G, t=2)
                    nc.vector.tensor_add(out=ov[:, 0], in0=sp[:, :, :, 0], in1=sp[:, :, :, 1])
                    nc.vector.tensor_add(out=ov[:, 1], in0=dp[:, :, :, 0], in1=dp[:, :, :, 1])
                    nc.vector.tensor_sub(out=ov[:, 2], in0=sp[:, :, :, 0], in1=sp[:, :, :, 1])
                    nc.vector.tensor_sub(out=ov[:, 3], in0=dp[:, :, :, 0], in1=dp[:, :, :, 1])
                    for q in range(4):
                        nc.sync.dma_start(out=yb[:, q, c0 : c0 + G, :], in_=ov[:, q])
    nc.compile()
    return nc


def kernel(x: np.ndarray) -> np.ndarray:
    from concourse.bass_utils import run_bass_kernel_spmd

    x = np.ascontiguousarray(np.asarray(x, dtype=np.float32))
    assert x.shape == (FULL_B, C, H, W), x.shape
    nc = _build_bass()
    shards = np.split(x, N_CORES, axis=0)
    in_maps = [{"x": s} for s in shards]
    res = run_bass_kernel_spmd(nc, in_maps, list(range(N_CORES)))
    return np.concatenate([r["y"] for r in res.results], axis=0)
